# revision 1
# baseline (speedup 1.0000x reference)
"""Trainium2 Bass kernel for nn_CDCL_47906065219864 (semi-supervised
segmentation loss: 3-layer conv extractor + classifier/projector heads +
CE/entropy/consistency/contrastive terms -> scalar loss).

Sharding (8 cores, uniform SPMD program, per-core data):
  core c: image b = c % 4, half = c // 4 (0 = top, 1 = bottom of feature map).
  Each core runs the extractor on its supervised half-image (with conv halo)
  and its unlabeled half-image, computes the sup-CE / entropy / consistency
  partial sums for its half, L2-normalizes its 800 projector pixels (= 8
  patches of anchors), AllGathers the student feature bank across cores,
  and computes the contrastive term for its own 800 anchors against the
  full 12800-entry bank (gathered student half + EMA half).
  NUM_CLASSES=2 lets every classifier quantity collapse to the logit
  difference d = (Wc[1]-Wc[0])@fea + (bc[1]-bc[0]):
     nll = softplus(d) - label*d,  H = softplus(d) - d*sigmoid(d),
     pseudo_label = (d > 0).
  Contrastive per anchor a (labels binary): with s = (a@bank)/TEMP,
     sum_b pos*logp = (1-la)*[2(T-T1) - (M-N1)L] + la*[2 T1 - N1 L],
  where T = a@Bsum, T1 = a@B1sum (bank sums, 2 tiny matmuls), and
  L = logsumexp_b s = SHIFT + log sum exp(s - SHIFT).
Host only shards/reshapes inputs (im2col for conv1, masks, EMA argmax)
and sums the 8 cores' partial-loss vectors into the final scalar.
"""

import os
import numpy as np

F = np.float32

# ---------------- geometry tables ----------------
# local row counts (uniform across cores; halo baked into host slicing)
SUP_NL1, SUP_NL2, SUP_NF = 87, 43, 21
UL_NL1, UL_NL2, UL_NF = 85, 41, 20
SUP_L1START = (0, 76)   # per half
UL_L1START = (0, 80)
SUP_FSTART = (0, 19)
UL_FSTART = (0, 20)

SHIFT = 16.0
INV_TEMP = 2.0
M_BANK = 12800.0

_CACHE = {}


# ---------------- host-side prep ----------------

def _resize_mat(oh, ih):
    Mx = np.zeros((oh, ih), F)
    s = np.linspace(0.0, ih - 1.0, oh)
    y0 = np.floor(s).astype(int)
    y1 = np.minimum(y0 + 1, ih - 1)
    w = (s - y0).astype(F)
    for i in range(oh):
        Mx[i, y0[i]] += 1 - w[i]
        Mx[i, y1[i]] += w[i]
    return Mx


def _im2col(img, l1start, n_l1):
    """img [3,320,320] -> [27, n_l1*160] for conv1 (stride2, SAME: pad 1 br)."""
    xp = np.zeros((3, 2 * (l1start + n_l1) + 2, 322), F)
    h = min(320, xp.shape[1])
    xp[:, :h, :320] = img[:, :h]
    out = np.empty((27, n_l1, 160), F)
    for c in range(3):
        for dy in range(3):
            for dx in range(3):
                sub = xp[c, 2 * l1start + dy: 2 * l1start + dy + 2 * n_l1:2,
                         dx: dx + 320:2]
                out[c * 9 + dy * 3 + dx] = sub
    return out.reshape(27, n_l1 * 160)


def _prep(inputs):
    x_l = np.ascontiguousarray(np.asarray(inputs['x_l'], F))
    y_l = np.asarray(inputs['y_l'])
    x_ul = np.ascontiguousarray(np.asarray(inputs['x_ul'], F))
    proj_ema = np.ascontiguousarray(np.asarray(inputs['proj_ul_ema'], F))
    z_ema = np.asarray(inputs['z_ul_ema'], F)
    W1 = np.asarray(inputs['W1'], F); W2 = np.asarray(inputs['W2'], F)
    W3 = np.asarray(inputs['W3'], F)
    Wc = np.asarray(inputs['Wc'], F)[:, :, 0, 0]
    bc = np.asarray(inputs['bc'], F)
    Wp = np.asarray(inputs['Wp'], F)[:, :, 0, 0]

    Ry = _resize_mat(320, 40)
    Rx = _resize_mat(320, 40)
    wc_d = Wc[1] - Wc[0]
    bc_d = F(bc[1] - bc[0])

    # EMA prep (input-only)
    pl_ema = np.argmax(z_ema, axis=1).astype(F)                # [4,40,40]
    e = z_ema - z_ema.max(axis=1, keepdims=True)
    p_ema = np.exp(e) / np.exp(e).sum(axis=1, keepdims=True)
    mask_ema = (p_ema.max(axis=1) > 0.6).astype(F)             # [4,40,40]
    labE = pl_ema.reshape(4, 1600)
    ebank = np.concatenate([proj_ema[b].reshape(128, 1600) for b in range(4)], axis=1)
    labEf = np.concatenate([labE[b] for b in range(4)])
    epack = np.stack([ebank.sum(1), (ebank * labEf[None]).sum(1)], axis=1).astype(F)
    en1 = np.array([[labEf.sum()]], F)

    w1m = np.ascontiguousarray(W1.transpose(1, 2, 3, 0).reshape(27, 64))
    w2t = np.ascontiguousarray(W2.transpose(1, 2, 3, 0).reshape(64, 9 * 128))
    w3t = np.ascontiguousarray(W3.transpose(1, 2, 3, 0).reshape(128, 9 * 256))
    wpt = np.ascontiguousarray(Wp.T.reshape(2, 128, 128).transpose(1, 0, 2).reshape(128, 256))
    wcd = np.ascontiguousarray(wc_d.reshape(2, 128).T)         # [128,2]
    bcd = np.array([[bc_d]], F)
    rxt = np.ascontiguousarray(Rx.T)                           # [40,320]

    shared = dict(w1m=w1m, w2t=w2t, w3t=w3t, wpt=wpt, wcd=wcd, bcd=bcd,
                  rxt=rxt, ebank=np.ascontiguousarray(ebank), epack=epack, en1=en1)

    in_maps = []
    for c in range(8):
        b, half = c % 4, c // 4
        xs = _im2col(x_l[b], SUP_L1START[half], SUP_NL1)
        xu = _im2col(x_ul[b], UL_L1START[half], UL_NL1)
        f0 = SUP_FSTART[half]
        ryt = np.ascontiguousarray(
            Ry[160 * half:160 * half + 160, f0:f0 + SUP_NF].T)  # [21,160]
        y_h = y_l[b, 160 * half:160 * half + 160]                # [160,320]
        vmask = (y_h != 255).astype(F)
        yf = np.clip(y_h, 0, 1).astype(F)
        supA = vmask
        supB = yf * vmask
        u0 = UL_FSTART[half]
        plm = (pl_ema[b, u0:u0 + 20] * mask_ema[b, u0:u0 + 20]).reshape(100, 8)
        mkm = mask_ema[b, u0:u0 + 20].reshape(100, 8)
        m = dict(shared)
        m.update(xs=xs, xu=xu, ryt=ryt, supA=np.ascontiguousarray(supA),
                 supB=np.ascontiguousarray(supB),
                 plm=np.ascontiguousarray(plm), mkm=np.ascontiguousarray(mkm))
        in_maps.append(m)

    meta = dict(sup_cnt=float((y_l != 255).sum()),
                cons_cnt=float(mask_ema.sum()),
                epoch=int(np.asarray(inputs['epoch'])))
    return in_maps, meta


def _combine(rows, meta):
    s = np.asarray(rows, np.float64).sum(axis=0)
    # slots 0/1/4/6 accumulate q = ln(sigmoid(-z)) = -softplus(z) products
    sup_nll = -(s[0] + s[1]) - (s[2] + s[3])
    ent = -s[4] - s[5]
    cons = -s[6] - s[7]
    contr_num, inc_sum = s[12], s[13]
    loss_sup = sup_nll / max(meta['sup_cnt'], 1.0)
    epoch = meta['epoch']
    if epoch < 5:
        return np.float32(loss_sup)
    loss_ent = ent / 6400.0
    loss_cons = cons / max(meta['cons_cnt'], 1.0)
    loss_contr = contr_num / max(inc_sum, 1.0)
    ramp = min(max(epoch / 40.0, 0.0), 1.0)
    cons_w = 1.0 * float(np.exp(-5.0 * (1.0 - ramp) ** 2))
    return np.float32(loss_sup + 0.1 * loss_contr + cons_w * loss_cons
                      + 0.01 * loss_ent)


# ---------------- bass program ----------------

def _build():
    import concourse.bacc as bacc
    import concourse.bass as bass
    import concourse.mybir as mybir
    from concourse import tile

    dt = mybir.dt
    F32 = dt.float32
    AF = mybir.ActivationFunctionType
    OP = mybir.AluOpType

    mmdt = os.environ.get('K_MMDT', 'f32')
    MDT = dt.float32r if mmdt == 'f32r' else dt.float32

    def bx(ap):
        return ap.bitcast(MDT) if MDT != F32 else ap

    nc = bacc.Bacc("TRN2", target_bir_lowering=False, debug=False,
                   num_devices=8)

    # ---- dram I/O ----
    din = {}
    for name, shape in [
            ('xs', [27, SUP_NL1 * 160]), ('xu', [27, UL_NL1 * 160]),
            ('ryt', [21, 160]), ('rxt', [40, 320]),
            ('supA', [160, 320]), ('supB', [160, 320]),
            ('plm', [100, 8]), ('mkm', [100, 8]),
            ('w1m', [27, 64]), ('w2t', [64, 9 * 128]), ('w3t', [128, 9 * 256]),
            ('wpt', [128, 256]), ('wcd', [128, 2]), ('bcd', [1, 1]),
            ('ebank', [128, 6400]), ('epack', [128, 2]), ('en1', [1, 1])]:
        din[name] = nc.dram_tensor(name, shape, F32, kind="ExternalInput")
    dout = nc.dram_tensor('part', [1, 16], F32, kind="ExternalOutput")

    eye_np = np.eye(128, dtype=np.float32)
    ones_np = np.ones((128, 128), dtype=np.float32)
    eye_d = nc.inline_tensor(eye_np, name='eye_c')
    ones_d = nc.inline_tensor(ones_np, name='ones_c')

    from contextlib import ExitStack
    with tile.TileContext(nc) as tc, ExitStack() as _es:
        cpool = _es.enter_context(tc.tile_pool(name="consts", bufs=1))
        big = _es.enter_context(tc.tile_pool(name="big", bufs=1))
        work = _es.enter_context(tc.tile_pool(name="work", bufs=1))
        wk2 = _es.enter_context(tc.tile_pool(name="wk2", bufs=2))
        dram = _es.enter_context(tc.tile_pool(name="dram", bufs=1, space="DRAM"))
        smallps = _es.enter_context(tc.tile_pool(name="smallps", bufs=2, space="PSUM"))

        # ---- consts / weights to SBUF ----
        def load(name):
            src = din[name]
            t = cpool.tile(list(src.shape), F32, name=f"{name}_sb")
            nc.sync.dma_start(t[:], src[:])
            return t

        eye_sb = cpool.tile([128, 128], F32, name="eye_sb")
        nc.sync.dma_start(eye_sb[:], eye_d[:])
        ones_sb = cpool.tile([128, 128], F32, name="ones_sb")
        nc.sync.dma_start(ones_sb[:], ones_d[:])

        w1sb = load('w1m')
        w2sb = load('w2t')      # [64, tap*128]
        w3sb = load('w3t')     # [128, tap*256]
        wpsb = load('wpt')     # [128, half*128]
        wcsb = load('wcd')
        bcdsb = load('bcd')
        rxsb = load('rxt')
        rysb = load('ryt')
        epsb = load('epack')
        en1sb = load('en1')
        plsb = load('plm')
        mksb = load('mkm')

        acc = work.tile([128, 12], F32, name="acc")
        nc.vector.memset(acc[:], 0.0)
        shiftsb = cpool.tile([128, 1], F32, name="shiftsb")
        nc.vector.memset(shiftsb[:], -SHIFT)

        # ================= conv stack =================
        def conv_stack(xdram, n_l1, n_l2, n_fea, tagp):
            with tc.tile_pool(name=f"convps{tagp}", bufs=6, space="PSUM") as cps:
                npix = n_l1 * 160
                l1t = big.tile([64, n_l1, 161], F32, tag="l1", name=f"l1{tagp}")
                nc.vector.memset(l1t[:, :, 160:161], 0.0)
                # conv1: stream im2col x through rotating chunk tiles
                xb = 0
                while xb < npix:
                    xn = min(3840, npix - xb)
                    xt = big.tile([27, 3840], F32, tag="x", name=f"x{tagp}",
                                  bufs=2)
                    nc.sync.dma_start(xt[:, :xn], xdram[:, xb:xb + xn])
                    c0 = 0
                    while c0 < xn:
                        n = min(480, xn - c0)
                        nr = n // 160
                        g0 = (xb + c0) // 160
                        ps = cps.tile([64, 480], F32, tag="cps", name="ps1")
                        nc.tensor.matmul(ps[:, :n], bx(w1sb[:]),
                                         bx(xt[:, c0:c0 + n]),
                                         start=True, stop=True)
                        nc.vector.tensor_scalar_max(
                            l1t[:, g0: g0 + nr, 0:160],
                            ps[:, :n].rearrange("p (r x) -> p r x", x=160), 0.0)
                        c0 += n
                    xb += xn
                # conv2: 6-row chunks, 9 taps accumulate
                l2t = big.tile([128, n_l2, 81], F32, tag="l2", name=f"l2{tagp}")
                nc.vector.memset(l2t[:, :, 80:81], 0.0)
                r0 = 0
                while r0 < n_l2:
                    nr = min(6, n_l2 - r0)
                    ps = cps.tile([128, 480], F32, tag="cps", name="ps2")
                    pv = ps[:, :nr * 80].rearrange("p (r x) -> p r x", x=80)
                    for tap in range(9):
                        dy, dxx = tap // 3, tap % 3
                        rhs = l1t[:, 2 * r0 + dy: 2 * r0 + dy + 2 * nr - 1:2,
                                  dxx: dxx + 159:2]
                        nc.tensor.matmul(pv, bx(w2sb[:, tap * 128:(tap + 1) * 128]),
                                         bx(rhs), start=(tap == 0), stop=(tap == 8))
                    nc.vector.tensor_scalar_max(l2t[:, r0:r0 + nr, 0:80], pv, 0.0)
                    r0 += nr
                # conv3: 12-row chunks, 2 M-halves, 9 taps
                feas = []
                for h in range(2):
                    ft = big.tile([128, n_fea * 40], F32, tag=f"fea{tagp}{h}",
                                  name=f"fea{tagp}{h}")
                    r0 = 0
                    while r0 < n_fea:
                        nr = min(12, n_fea - r0)
                        ps = cps.tile([128, 480], F32, tag="cps", name="ps3")
                        pv = ps[:, :nr * 40].rearrange("p (r x) -> p r x", x=40)
                        for tap in range(9):
                            dy, dxx = tap // 3, tap % 3
                            rhs = l2t[:, 2 * r0 + dy: 2 * r0 + dy + 2 * nr - 1:2,
                                      dxx: dxx + 79:2]
                            nc.tensor.matmul(
                                pv, bx(w3sb[:, tap * 256 + h * 128:
                                            tap * 256 + h * 128 + 128]),
                                bx(rhs), start=(tap == 0), stop=(tap == 8))
                        nc.vector.tensor_scalar_max(
                            ft[:, r0 * 40:(r0 + nr) * 40],
                            ps[:, :nr * 40], 0.0)
                        r0 += nr
                    feas.append(ft)
            return feas

        with nc.named_scope("conv_sup"):
            fs_lo, fs_hi = conv_stack(din['xs'], SUP_NL1, SUP_NL2, SUP_NF, "s")
        with nc.named_scope("conv_ul"):
            fu_lo, fu_hi = conv_stack(din['xu'], UL_NL1, UL_NL2, UL_NF, "u")

        # ================= ul projection + payload + AllGather =================
        with nc.named_scope("proj"):
            proj_raw = work.tile([128, 800], F32, name="proj_raw")
            fuv = [t[:].rearrange("p (y x) -> p y x", x=40) for t in (fu_lo, fu_hi)]
            for ci in range(2):
                psp = smallps.tile([128, 400], F32, tag="sp", name="psp")
                for pp in range(4):
                    p = ci * 4 + pp
                    jl, kk = p // 4, p % 4
                    for h in range(2):
                        nc.tensor.matmul(
                            psp[:, pp * 100:(pp + 1) * 100],
                            bx(wpsb[:, h * 128:(h + 1) * 128]),
                            bx(fuv[h][:, 10 * jl:10 * jl + 10,
                                      10 * kk:10 * kk + 10]),
                            start=(h == 0), stop=(h == 1))
                nc.scalar.copy(proj_raw[:, ci * 400:(ci + 1) * 400], psp[:])
            # d_ul row + labels
            du_row = work.tile([1, 800], F32, name="du_row")
            for ci in range(2):
                psd = smallps.tile([1, 400], F32, tag="sp", name="psd")
                for h in range(2):
                    nc.tensor.matmul(psd[:], bx(wcsb[:, h:h + 1]),
                                     bx((fu_lo, fu_hi)[h][:, ci * 400:(ci + 1) * 400]),
                                     start=(h == 0), stop=(h == 1))
                nc.vector.tensor_scalar(du_row[:, ci * 400:(ci + 1) * 400],
                                        psd[:], bcdsb[0:1, 0:1], None,
                                        op0=OP.add)
            la_row = work.tile([1, 800], F32, name="la_row")
            nc.vector.tensor_scalar(la_row[:], du_row[:], 0.0, None, op0=OP.is_gt)
            la_pat = work.tile([1, 800], F32, name="la_pat")
            lrv = la_row[:].rearrange("p (y x) -> p y x", x=40)
            for p in range(8):
                jl, kk = p // 4, p % 4
                nc.sync.dma_start(
                    la_pat[:, p * 100:(p + 1) * 100].rearrange(
                        "p (y x) -> p y x", x=10),
                    lrv[:, 10 * jl:10 * jl + 10, 10 * kk:10 * kk + 10])
            # normalize proj
            sq = work.tile([128, 800], F32, name="sq")
            nc.vector.tensor_mul(sq[:], proj_raw[:], proj_raw[:])
            nrm = work.tile([1, 800], F32, name="nrm")
            for ci in range(2):
                pss = smallps.tile([1, 400], F32, tag="sp", name="pss")
                nc.tensor.matmul(pss[:], bx(ones_sb[:, 0:1]),
                                 bx(sq[:, ci * 400:(ci + 1) * 400]),
                                 start=True, stop=True)
                nc.scalar.activation(nrm[:, ci * 400:(ci + 1) * 400], pss[:],
                                     AF.Sqrt)
            nc.vector.tensor_scalar_max(nrm[:], nrm[:], 1e-12)
            inv = work.tile([1, 800], F32, name="inv")
            nc.vector.reciprocal(inv[:], nrm[:])
            inv_bc = work.tile([128, 800], F32, tag="bc800", name="inv_bc")
            for ci in range(2):
                psb1 = smallps.tile([128, 400], F32, tag="sp", name="psb1")
                nc.tensor.matmul(psb1[:], bx(ones_sb[0:1, :]),
                                 bx(inv[:, ci * 400:(ci + 1) * 400]),
                                 start=True, stop=True)
                nc.scalar.copy(inv_bc[:, ci * 400:(ci + 1) * 400], psb1[:])
            proj_n = work.tile([128, 800], F32, name="proj_n")
            nc.vector.tensor_mul(proj_n[:], proj_raw[:], inv_bc[:])
            la_bc = work.tile([128, 800], F32, tag="bc800", name="la_bc")
            for ci in range(2):
                psb2 = smallps.tile([128, 400], F32, tag="sp", name="psb2")
                nc.tensor.matmul(psb2[:], bx(ones_sb[0:1, :]),
                                 bx(la_pat[:, ci * 400:(ci + 1) * 400]),
                                 start=True, stop=True)
                nc.scalar.copy(la_bc[:, ci * 400:(ci + 1) * 400], psb2[:])
            vsum = work.tile([128, 1], F32, name="vsum")
            nc.vector.tensor_reduce(vsum[:], proj_n[:], mybir.AxisListType.X, OP.add)
            v1 = work.tile([128, 1], F32, name="v1")
            nc.vector.scalar_tensor_tensor(sq[:], proj_n[:], 1.0, la_bc[:],
                                           op0=OP.mult, op1=OP.mult, accum_out=v1[:])
            n1loc = work.tile([1, 1], F32, name="n1loc")
            nc.vector.tensor_reduce(n1loc[:], la_row[:], mybir.AxisListType.X, OP.add)

            pay = dram.tile([128, 808], F32, name="pay")
            gath = dram.tile([1024, 808], F32, name="gath", addr_space="Shared")
            zpad = cpool.tile([128, 6], F32, name="zpad")
            nc.vector.memset(zpad[:], 0.0)
            nc.sync.dma_start(pay[:, 802:808], zpad[:])
            nc.sync.dma_start(pay[:, 0:800], proj_n[:])
            nc.sync.dma_start(pay[:, 800:801], vsum[:])
            nc.sync.dma_start(pay[:, 801:802], v1[:])
            nc.sync.dma_start(pay[0:1, 802:803], n1loc[:])
            nc.gpsimd.collective_compute(
                "AllGather", OP.bypass,
                replica_groups=[list(range(8))],
                ins=[pay[:].opt()], outs=[gath[:].opt()])

        # ================= sup head (overlaps AllGather) =================
        with nc.named_scope("sup_head"):
            dsup = work.tile([1, 840], F32, name="dsup")
            for ci in range(2):
                psd2 = smallps.tile([1, 420], F32, tag="sp", name="psd2")
                for h in range(2):
                    nc.tensor.matmul(psd2[:], bx(wcsb[:, h:h + 1]),
                                     bx((fs_lo, fs_hi)[h][:, ci * 420:(ci + 1) * 420]),
                                     start=(h == 0), stop=(h == 1))
                nc.vector.tensor_scalar(dsup[:, ci * 420:(ci + 1) * 420],
                                        psd2[:], bcdsb[0:1, 0:1], None,
                                        op0=OP.add)
            d_yx = work.tile([21, 40], F32, name="d_yx")
            nc.sync.dma_start(d_yx[:], dsup[:].rearrange("p (y x) -> p y x", x=40))
            pstr = smallps.tile([40, 21], F32, tag="sp", name="pstr")
            nc.tensor.transpose(pstr[:], d_yx[:], eye_sb[0:21, 0:21])
            dT = work.tile([40, 21], F32, name="dT")
            nc.scalar.copy(dT[:], pstr[:])
            pst1 = smallps.tile([21, 320], F32, tag="sp", name="pst1")
            nc.tensor.matmul(pst1[:], bx(dT[:]), bx(rxsb[:]), start=True, stop=True)
            tmp1 = work.tile([21, 320], F32, name="tmp1")
            nc.scalar.copy(tmp1[:], pst1[:])
            # z chunks + CE partials
            for ci, (p0, npp) in enumerate([(0, 128), (128, 32)]):
                psz = smallps.tile([128, 320], F32, tag="sp", name="psz")
                nc.tensor.matmul(psz[:npp, :], bx(rysb[:, p0:p0 + npp]),
                                 bx(tmp1[:]), start=True, stop=True)
                asb = wk2.tile([128, 320], F32, tag="ab", name="asb")
                bsb = wk2.tile([128, 320], F32, tag="ab", name="bsb")
                nc.sync.dma_start(asb[:npp, :], din['supA'][p0:p0 + npp, :])
                nc.sync.dma_start(bsb[:npp, :], din['supB'][p0:p0 + npp, :])
                sp = wk2.tile([128, 320], F32, tag="sp2", name="spz")
                nc.scalar.activation(sp[:npp, :], psz[:npp, :], AF.Sigmoid,
                                     scale=-1.0)
                nc.scalar.activation(sp[:npp, :], sp[:npp, :], AF.Ln)
                jk = wk2.tile([128, 320], F32, tag="jk", name="jk")
                nc.vector.scalar_tensor_tensor(
                    jk[:npp, :], sp[:npp, :], 1.0, asb[:npp, :],
                    op0=OP.mult, op1=OP.mult,
                    accum_out=acc[0:npp, 0 + ci:1 + ci])
                nc.vector.scalar_tensor_tensor(
                    jk[:npp, :], psz[:npp, :], 1.0, bsb[:npp, :],
                    op0=OP.mult, op1=OP.mult,
                    accum_out=acc[0:npp, 2 + ci:3 + ci])

        # ================= ul head =================
        with nc.named_scope("ul_head"):
            dut = work.tile([100, 8], F32, name="dut")
            nc.sync.dma_start(dut[:], du_row[:].rearrange("p (a b) -> p a b", b=8))
            spu = work.tile([100, 8], F32, name="spu")
            nc.scalar.activation(spu[:], dut[:], AF.Sigmoid, scale=-1.0)
            nc.scalar.activation(spu[:], spu[:], AF.Ln,
                                 accum_out=acc[0:100, 4:5])
            sgu = work.tile([100, 8], F32, name="sgu")
            nc.scalar.activation(sgu[:], dut[:], AF.Sigmoid)
            jk2 = work.tile([100, 8], F32, name="jk2")
            nc.vector.scalar_tensor_tensor(jk2[:], dut[:], 1.0, sgu[:],
                                           op0=OP.mult, op1=OP.mult,
                                           accum_out=acc[0:100, 5:6])
            nc.vector.scalar_tensor_tensor(jk2[:], spu[:], 1.0, mksb[:],
                                           op0=OP.mult, op1=OP.mult,
                                           accum_out=acc[0:100, 6:7])
            nc.vector.scalar_tensor_tensor(jk2[:], dut[:], 1.0, plsb[:],
                                           op0=OP.mult, op1=OP.mult,
                                           accum_out=acc[0:100, 7:8])
            # per-patch anchor labels [100, 8] and patch label sums [1, 8]
            la_t8 = work.tile([100, 8], F32, name="la_t8")
            for p in range(8):
                psl = smallps.tile([100, 1], F32, tag="sp", name="psl")
                nc.tensor.matmul(psl[:],
                                 bx(la_pat[:, p * 100:(p + 1) * 100]),
                                 bx(ones_sb[0:1, 0:1]), start=True, stop=True)
                nc.scalar.copy(la_t8[:, p:p + 1], psl[:])
            psps_ = smallps.tile([1, 8], F32, tag="sp", name="psps")
            nc.tensor.matmul(psps_[:], bx(ones_sb[0:100, 0:1]), bx(la_t8[:]),
                             start=True, stop=True)
            ps8 = work.tile([1, 8], F32, name="ps8")
            nc.scalar.copy(ps8[:], psps_[:])

        # ================= post-AG bank assembly =================
        with nc.named_scope("bank"):
            bank = big.tile([128, 12800], dt.bfloat16, tag="l1", name="bank")
            sbank = bank[:, 0:6400]
            ebsb = bank[:, 6400:12800]
            for i in range(4):
                nc.gpsimd.dma_start(ebsb[:, i * 1600:(i + 1) * 1600],
                                    din['ebank'][:, i * 1600:(i + 1) * 1600])
            BB = work.tile([128, 2], F32, name="BB")
            nc.vector.tensor_copy(BB[:], epsb[:])
            n1t = work.tile([1, 1], F32, name="n1t")
            nc.vector.tensor_copy(n1t[:], en1sb[:])
            for r in range(8):
                nc.gpsimd.dma_start(sbank[:, r * 800:(r + 1) * 800],
                                    gath[r * 128:(r + 1) * 128, 0:800])
                vv = wk2.tile([128, 2], F32, tag="vv", name="vv")
                nc.sync.dma_start(vv[:], gath[r * 128:(r + 1) * 128, 800:802])
                nc.vector.tensor_add(BB[:], BB[:], vv[:])
                nn = wk2.tile([1, 1], F32, tag="nn", name="nn")
                nc.sync.dma_start(nn[:], gath[r * 128:r * 128 + 1, 802:803])
                nc.vector.tensor_add(n1t[:], n1t[:], nn[:])
            psn = smallps.tile([100, 1], F32, tag="sp", name="psn")
            nc.tensor.matmul(psn[:], bx(ones_sb[0:1, 0:100]), bx(n1t[:]),
                             start=True, stop=True)
            N1b = work.tile([100, 1], F32, name="N1b")
            nc.scalar.copy(N1b[:], psn[:])
            MN1 = work.tile([100, 1], F32, name="MN1")
            nc.vector.tensor_scalar(MN1[:], N1b[:], -1.0, M_BANK,
                                    op0=OP.mult, op1=OP.add)
            DN = work.tile([100, 1], F32, name="DN")
            nc.vector.tensor_scalar(DN[:], N1b[:], 2.0, -M_BANK,
                                    op0=OP.mult, op1=OP.add)

        # ================= contrastive =================
        with nc.named_scope("contrastive"), \
                tc.tile_pool(name="cps", bufs=6, space="PSUM") as cps2:
            PA = work.tile([100, 8], F32, name="PA")
            proj_nb = work.tile([128, 800], dt.bfloat16, name="proj_nb")
            nc.vector.tensor_copy(proj_nb[:], proj_n[:])
            BBb = work.tile([128, 2], dt.bfloat16, name="BBb")
            nc.vector.tensor_copy(BBb[:], BB[:])
            chunks = []
            for half_t, nh in ((sbank, 6400), (ebsb, 6400)):
                c0 = 0
                while c0 < nh:
                    n = min(512, nh - c0)
                    chunks.append((half_t, c0, n))
                    c0 += n
            for p in range(8):
                A_p = proj_nb[:, p * 100:(p + 1) * 100]
                psT = smallps.tile([100, 2], F32, tag="sp", name="psT")
                nc.tensor.matmul(psT[:], A_p, BBb[:], start=True, stop=True)
                TT = wk2.tile([100, 2], F32, tag="tt", name="TT")
                nc.scalar.copy(TT[:], psT[:])
                zc = wk2.tile([100, 26], F32, tag="zc", name="zc")
                for i, (bt, c0, n) in enumerate(chunks):
                    ps_s = cps2.tile([100, 512], F32, tag="cs", name="ps_s")
                    nc.tensor.matmul(ps_s[:, :n], A_p, bt[:, c0:c0 + n],
                                     start=True, stop=True)
                    sc = wk2.tile([100, 512], F32, tag="sc", name="sc", bufs=4)
                    nc.scalar.activation(sc[:, :n], ps_s[:, :n], AF.Exp,
                                         scale=INV_TEMP,
                                         bias=shiftsb[0:100, 0:1],
                                         accum_out=zc[:, i:i + 1])
                # assemble per-anchor loss (one tile, column slices)
                pv = wk2.tile([100, 12], F32, tag="pv", name="pv")
                Zs, Lv, m1, U1, tm, m0, U0, d10, pd, dd, rr = (
                    pv[:, i:i + 1] for i in range(11))
                nc.vector.tensor_reduce(Zs, zc[:], mybir.AxisListType.X, OP.add)
                nc.scalar.activation(Lv, Zs, AF.Ln)
                nc.vector.tensor_scalar_add(Lv, Lv, SHIFT)
                nc.vector.tensor_mul(m1, N1b[:], Lv)
                nc.vector.scalar_tensor_tensor(U1, TT[:, 1:2], INV_TEMP, m1,
                                               op0=OP.mult, op1=OP.subtract)
                nc.vector.tensor_sub(tm, TT[:, 0:1], TT[:, 1:2])
                nc.vector.tensor_mul(m0, MN1[:], Lv)
                nc.vector.scalar_tensor_tensor(U0, tm, INV_TEMP, m0,
                                               op0=OP.mult, op1=OP.subtract)
                nc.vector.tensor_sub(d10, U1, U0)
                nc.vector.scalar_tensor_tensor(pd, la_t8[:, p:p + 1], 1.0,
                                               d10, op0=OP.mult, op1=OP.mult)
                nc.vector.tensor_add(pd, pd, U0)
                nc.vector.scalar_tensor_tensor(dd, la_t8[:, p:p + 1], 1.0,
                                               DN[:], op0=OP.mult, op1=OP.mult)
                nc.vector.tensor_add(dd, dd, MN1[:])
                nc.vector.reciprocal(rr, dd)
                nc.vector.tensor_mul(PA[:, p:p + 1], pd, rr)

            pspl = smallps.tile([1, 8], F32, tag="sp", name="pspl")
            nc.tensor.matmul(pspl[:], bx(ones_sb[0:100, 0:1]), bx(PA[:]),
                             start=True, stop=True)
            pl8 = work.tile([1, 8], F32, name="pl8")
            nc.scalar.activation(pl8[:], pspl[:], AF.Copy, scale=-0.01)
            fc = work.tile([1, 8], F32, name="fc")
            nc.vector.tensor_scalar_mul(fc[:], ps8[:], 0.01)
            g1 = work.tile([1, 8], F32, name="g1")
            nc.vector.tensor_scalar(g1[:], fc[:], 0.1, None, op0=OP.is_gt)
            g2 = work.tile([1, 8], F32, name="g2")
            nc.vector.tensor_scalar(g2[:], fc[:], 0.9, None, op0=OP.is_lt)
            inc = work.tile([1, 8], F32, name="inc")
            nc.vector.tensor_mul(inc[:], g1[:], g2[:])
            nc.vector.tensor_scalar(inc[:], inc[:], -1.0, 1.0,
                                    op0=OP.mult, op1=OP.add)
            cn = work.tile([1, 8], F32, name="cn")
            nc.vector.tensor_mul(cn[:], inc[:], pl8[:])
            outrow = work.tile([1, 16], F32, name="outrow")
            nc.vector.memset(outrow[:], 0.0)
            nc.vector.tensor_reduce(outrow[:, 12:13], cn[:],
                                    mybir.AxisListType.X, OP.add)
            nc.vector.tensor_reduce(outrow[:, 13:14], inc[:],
                                    mybir.AxisListType.X, OP.add)
            psacc = smallps.tile([1, 12], F32, tag="sp", name="psacc")
            nc.tensor.matmul(psacc[:], bx(ones_sb[:, 0:1]), bx(acc[:]),
                             start=True, stop=True)
            nc.scalar.copy(outrow[:, 0:12], psacc[:])
            nc.sync.dma_start(dout[:], outrow[:])

    nc.compile()
    return nc


def _get_nc():
    if 'nc' not in _CACHE:
        _CACHE['nc'] = _build()
    return _CACHE['nc']


def run_on_cores(inputs, trace=False):
    """Returns (scalar_loss, exec_time_ns_or_None)."""
    from concourse.bass_utils import run_bass_kernel_spmd
    in_maps, meta = _prep(inputs)
    nc = _get_nc()
    res = run_bass_kernel_spmd(nc, in_maps, core_ids=list(range(8)),
                               trace=trace)
    rows = [res.results[c]['part'][0] for c in range(8)]
    return _combine(rows, meta), res.exec_time_ns


def run_timed(inputs, reps=5):
    """Correctness + timing: jit once, pre-place inputs on devices, time
    repeated executions (min over reps approximates HW exec + dispatch)."""
    import time
    import jax
    import numpy as np_
    import concourse.mybir as mybir
    from jax.sharding import Mesh, PartitionSpec, NamedSharding
    from jax.experimental.shard_map import shard_map
    from concourse import bass2jax
    from concourse.bass2jax import _bass_exec_p, partition_id_tensor

    bass2jax.install_neuronx_cc_hook()
    in_maps, meta = _prep(inputs)
    nc = _get_nc()

    partition_name = nc.partition_id_tensor.name if nc.partition_id_tensor else None
    in_names, out_names, out_avals, zero_outs = [], [], [], []
    for alloc in nc.m.functions[0].allocations:
        if not isinstance(alloc, mybir.MemoryLocationSet):
            continue
        name = alloc.memorylocations[0].name
        if alloc.kind == "ExternalInput":
            if name != partition_name:
                in_names.append(name)
        elif alloc.kind == "ExternalOutput":
            out_names.append(name)
            shape = tuple(alloc.tensor_shape)
            dtype = mybir.dt.np(alloc.dtype)
            out_avals.append(jax.core.ShapedArray(shape, dtype))
            zero_outs.append(np_.zeros(shape, dtype))
    n_params = len(in_names)
    all_names = in_names + out_names + ([partition_name] if partition_name else [])

    def _body(*args):
        operands = list(args)
        if partition_name is not None:
            operands.append(partition_id_tensor())
        outs = _bass_exec_p.bind(
            *operands, out_avals=tuple(out_avals), in_names=tuple(all_names),
            out_names=tuple(out_names), lowering_input_output_aliases=(),
            sim_require_finite=True, sim_require_nnan=True, nc=nc)
        return tuple(outs)

    devices = jax.devices()[:8]
    mesh = Mesh(np_.asarray(devices), ("core",))
    spec = NamedSharding(mesh, PartitionSpec("core"))
    n_outs = len(out_names)
    sharded = jax.jit(
        shard_map(_body, mesh=mesh,
                  in_specs=(PartitionSpec("core"),) * (n_params + n_outs),
                  out_specs=(PartitionSpec("core"),) * n_outs,
                  check_rep=False),
        keep_unused=True)
    concat_in = [
        jax.device_put(np_.concatenate(
            [np_.asarray(in_maps[c][in_names[i]]) for c in range(8)], axis=0), spec)
        for i in range(n_params)]
    concat_zeros = [
        jax.device_put(np_.zeros((8 * z.shape[0], *z.shape[1:]), z.dtype), spec)
        for z in zero_outs]
    jax.block_until_ready(concat_in)

    times = []
    outs = None
    for _ in range(reps):
        t0 = time.perf_counter()
        outs = sharded(*concat_in, *concat_zeros)
        jax.block_until_ready(outs)
        times.append(time.perf_counter() - t0)
    oarr = np_.asarray(outs[out_names.index('part')]).reshape(8, *out_avals[0].shape)
    rows = [oarr[c][0] for c in range(8)]
    return _combine(rows, meta), times


def bench_slope(inputs, k_list=(1, 9), reps=6):
    """Device-time measurement: one jit dispatch runs the NEFF K times,
    serialized by threading call i's outputs into call i+1's output-buffer
    operands. slope = (wall[K2]-wall[K1])/(K2-K1) ~ true per-exec device time."""
    import time
    import jax
    import numpy as np_
    import concourse.mybir as mybir
    from jax.sharding import Mesh, PartitionSpec, NamedSharding
    from jax.experimental.shard_map import shard_map
    from concourse import bass2jax
    from concourse.bass2jax import _bass_exec_p, partition_id_tensor

    bass2jax.install_neuronx_cc_hook()
    in_maps, meta = _prep(inputs)
    nc = _get_nc()
    partition_name = nc.partition_id_tensor.name if nc.partition_id_tensor else None
    in_names, out_names, out_avals, zero_outs = [], [], [], []
    for alloc in nc.m.functions[0].allocations:
        if not isinstance(alloc, mybir.MemoryLocationSet):
            continue
        name = alloc.memorylocations[0].name
        if alloc.kind == "ExternalInput":
            if name != partition_name:
                in_names.append(name)
        elif alloc.kind == "ExternalOutput":
            out_names.append(name)
            shape = tuple(alloc.tensor_shape)
            dtype = mybir.dt.np(alloc.dtype)
            out_avals.append(jax.core.ShapedArray(shape, dtype))
            zero_outs.append(np_.zeros(shape, dtype))
    n_params = len(in_names)
    all_names = in_names + out_names + ([partition_name] if partition_name else [])
    devices = jax.devices()[:8]
    mesh = Mesh(np_.asarray(devices), ("core",))
    spec = NamedSharding(mesh, PartitionSpec("core"))
    concat_in = [
        jax.device_put(np_.concatenate(
            [np_.asarray(in_maps[c][in_names[i]]) for c in range(8)], axis=0), spec)
        for i in range(n_params)]
    concat_zeros = [
        jax.device_put(np_.zeros((8 * z.shape[0], *z.shape[1:]), z.dtype), spec)
        for z in zero_outs]
    jax.block_until_ready(concat_in)
    n_outs = len(out_names)

    results = {}
    for K in k_list:
        def _body(*args, K=K):
            ins = list(args[:n_params])
            z = list(args[n_params:])
            for _k in range(K):
                operands = ins + z
                if partition_name is not None:
                    operands.append(partition_id_tensor())
                z = list(_bass_exec_p.bind(
                    *operands, out_avals=tuple(out_avals),
                    in_names=tuple(all_names), out_names=tuple(out_names),
                    lowering_input_output_aliases=(),
                    sim_require_finite=True, sim_require_nnan=True, nc=nc))
            return tuple(z)
        sharded = jax.jit(
            shard_map(_body, mesh=mesh,
                      in_specs=(PartitionSpec("core"),) * (n_params + n_outs),
                      out_specs=(PartitionSpec("core"),) * n_outs,
                      check_rep=False), keep_unused=True)
        times = []
        outs = None
        for _ in range(reps):
            t0 = time.perf_counter()
            outs = sharded(*concat_in, *concat_zeros)
            jax.block_until_ready(outs)
            times.append(time.perf_counter() - t0)
        results[K] = (min(times), times)
        oarr = np_.asarray(outs[out_names.index('part')]).reshape(8, 16)
        results[(K, 'val')] = _combine([oarr[c] for c in range(8)], meta)
    ks = sorted(k_list)
    slope = (results[ks[-1]][0] - results[ks[0]][0]) / (ks[-1] - ks[0])
    return slope, results


def bench_chain_slope(inputs, n_small=32, n_big=256, reps=4):
    """Per-execution device time: chain N executions (each consumes the
    previous call's output buffers -> serialized on device), slope between
    n_small and n_big cancels the axon dispatch overhead."""
    import time
    import jax
    import numpy as np_
    import concourse.mybir as mybir
    from jax.sharding import Mesh, PartitionSpec, NamedSharding
    from jax.experimental.shard_map import shard_map
    from concourse import bass2jax
    from concourse.bass2jax import _bass_exec_p, partition_id_tensor

    bass2jax.install_neuronx_cc_hook()
    in_maps, meta = _prep(inputs)
    nc = _get_nc()
    pname = nc.partition_id_tensor.name if nc.partition_id_tensor else None
    in_names, out_names, out_avals, zero_outs = [], [], [], []
    for alloc in nc.m.functions[0].allocations:
        if not isinstance(alloc, mybir.MemoryLocationSet):
            continue
        name = alloc.memorylocations[0].name
        if alloc.kind == "ExternalInput":
            if name != pname:
                in_names.append(name)
        elif alloc.kind == "ExternalOutput":
            out_names.append(name)
            shape = tuple(alloc.tensor_shape)
            dtype = mybir.dt.np(alloc.dtype)
            out_avals.append(jax.core.ShapedArray(shape, dtype))
            zero_outs.append(np_.zeros(shape, dtype))
    n_params = len(in_names)
    all_names = in_names + out_names + ([pname] if pname else [])

    def _body(*args):
        operands = list(args)
        if pname:
            operands.append(partition_id_tensor())
        return tuple(_bass_exec_p.bind(
            *operands, out_avals=tuple(out_avals), in_names=tuple(all_names),
            out_names=tuple(out_names), lowering_input_output_aliases=(),
            sim_require_finite=True, sim_require_nnan=True, nc=nc))

    devices = jax.devices()[:8]
    mesh = Mesh(np_.asarray(devices), ("core",))
    spec = NamedSharding(mesh, PartitionSpec("core"))
    n_outs = len(out_names)
    sharded = jax.jit(shard_map(_body, mesh=mesh,
                                in_specs=(PartitionSpec("core"),) * (n_params + n_outs),
                                out_specs=(PartitionSpec("core"),) * n_outs,
                                check_rep=False), keep_unused=True)
    concat_in = [jax.device_put(np_.concatenate(
        [np_.asarray(in_maps[c][in_names[i]]) for c in range(8)], axis=0), spec)
        for i in range(n_params)]
    concat_zeros = [jax.device_put(
        np_.zeros((8 * z.shape[0], *z.shape[1:]), z.dtype), spec)
        for z in zero_outs]
    jax.block_until_ready(concat_in)

    def run_chain(N):
        z = list(concat_zeros)
        t0 = time.perf_counter()
        for _ in range(N):
            z = list(sharded(*concat_in, *z))
        jax.block_until_ready(z)
        return time.perf_counter() - t0

    run_chain(2)  # warm-up / compile
    mins = {}
    for N in (n_small, n_big):
        mins[N] = min(run_chain(N) for _ in range(reps))
    slope = (mins[n_big] - mins[n_small]) / (n_big - n_small)
    return slope, mins


def kernel(**inputs):
    out, _ = run_on_cores(inputs, trace=False)
    return out



# revision 10
# speedup vs baseline: 1.1277x; 1.1277x over previous
"""Trainium2 Bass kernel for nn_CDCL_47906065219864 (semi-supervised
segmentation loss: 3-layer conv extractor + classifier/projector heads +
CE/entropy/consistency/contrastive terms -> scalar loss).

Sharding (8 cores, uniform SPMD program, per-core data):
  core c: image b = c % 4, half = c // 4 (0 = top, 1 = bottom of feature map).
  Each core runs the extractor on its supervised half-image (with conv halo)
  and its unlabeled half-image, computes the sup-CE / entropy / consistency
  partial sums for its half, L2-normalizes its 800 projector pixels (= 8
  patches of anchors), AllGathers the student feature bank across cores
  (bf16 payload), and computes the contrastive term for its own 800 anchors
  against the full 12800-entry bank (gathered student half + EMA half from
  host input).

  NUM_CLASSES=2 lets every classifier quantity collapse to the logit
  difference d = (Wc[1]-Wc[0])@fea + (bc[1]-bc[0]):
     nll = softplus(d) - label*d,  H = softplus(d) - d*sigmoid(d),
     pseudo_label = (d > 0).
  All transcendentals are expressed through exp/ln only (softplus(d) =
  ln(1+e^d), sigmoid = t/(1+t) with t=e^d, 1/sqrt(x) = e^(-ln(x)/2)) so the
  Act engine loads a single table set (natural_log_exp_and_others) once.

  Contrastive per anchor a (labels binary): with s = (a@bank)/TEMP,
     sum_b pos*logp = (1-la)*[2(T-T1) - (M-N1)L] + la*[2 T1 - N1 L],
  where T = a@Bsum, T1 = a@B1sum (bank sums, one tiny matmul), and
  L = logsumexp_b s = SHIFT + log sum exp(s - SHIFT). Score chunks put 128
  bank entries on partitions x 800 anchors on the free dim; exp() on Act;
  the per-anchor sum over bank entries is a ones-vector matmul accumulated
  across all 100 chunks in PSUM. The 50 EMA chunks only need host data and
  run while the AllGather is in flight.

Convs run in bf16 (inputs/weights converted on host, relus write bf16),
which is 4x the fp32 matmul rate on PE. Host only shards/reshapes inputs
(im2col for conv1, masks, EMA argmax) and sums the 8 cores' partial-loss
vectors into the final scalar.
"""

import numpy as np
import ml_dtypes

F = np.float32
BFH = ml_dtypes.bfloat16

# ---------------- geometry tables ----------------
# local row counts (uniform across cores; halo baked into host slicing)
SUP_NL1, SUP_NL2, SUP_NF = 87, 43, 21
UL_NL1, UL_NL2, UL_NF = 85, 41, 20
SUP_L1START = (0, 76)   # per half
UL_L1START = (0, 80)
SUP_FSTART = (0, 19)
UL_FSTART = (0, 20)

SHIFT = 16.0
INV_TEMP = 2.0
M_BANK = 12800.0

_CACHE = {}


# ---------------- host-side prep ----------------

def _resize_mat(oh, ih):
    Mx = np.zeros((oh, ih), F)
    s = np.linspace(0.0, ih - 1.0, oh)
    y0 = np.floor(s).astype(int)
    y1 = np.minimum(y0 + 1, ih - 1)
    w = (s - y0).astype(F)
    for i in range(oh):
        Mx[i, y0[i]] += 1 - w[i]
        Mx[i, y1[i]] += w[i]
    return Mx


def _im2col(img, l1start, n_l1):
    """img [3,320,320] -> [27, n_l1*160] for conv1 (stride2, SAME: pad 1 br)."""
    xp = np.zeros((3, 2 * (l1start + n_l1) + 2, 322), F)
    h = min(320, xp.shape[1])
    xp[:, :h, :320] = img[:, :h]
    out = np.empty((27, n_l1, 160), F)
    for c in range(3):
        for dy in range(3):
            for dx in range(3):
                sub = xp[c, 2 * l1start + dy: 2 * l1start + dy + 2 * n_l1:2,
                         dx: dx + 320:2]
                out[c * 9 + dy * 3 + dx] = sub
    return out.reshape(27, n_l1 * 160)


def _prep(inputs):
    x_l = np.ascontiguousarray(np.asarray(inputs['x_l'], F))
    y_l = np.asarray(inputs['y_l'])
    x_ul = np.ascontiguousarray(np.asarray(inputs['x_ul'], F))
    proj_ema = np.ascontiguousarray(np.asarray(inputs['proj_ul_ema'], F))
    z_ema = np.asarray(inputs['z_ul_ema'], F)
    W1 = np.asarray(inputs['W1'], F); W2 = np.asarray(inputs['W2'], F)
    W3 = np.asarray(inputs['W3'], F)
    Wc = np.asarray(inputs['Wc'], F)[:, :, 0, 0]
    bc = np.asarray(inputs['bc'], F)
    Wp = np.asarray(inputs['Wp'], F)[:, :, 0, 0]

    Ry = _resize_mat(320, 40)
    Rx = _resize_mat(320, 40)
    wc_d = Wc[1] - Wc[0]
    bc_d = F(bc[1] - bc[0])

    # EMA prep (input-only)
    pl_ema = np.argmax(z_ema, axis=1).astype(F)                # [4,40,40]
    e = z_ema - z_ema.max(axis=1, keepdims=True)
    p_ema = np.exp(e) / np.exp(e).sum(axis=1, keepdims=True)
    mask_ema = (p_ema.max(axis=1) > 0.6).astype(F)             # [4,40,40]
    labE = pl_ema.reshape(4, 1600)
    ebank = np.concatenate([proj_ema[b].reshape(128, 1600) for b in range(4)], axis=1)
    labEf = np.concatenate([labE[b] for b in range(4)])
    epack = np.stack([ebank.sum(1), (ebank * labEf[None]).sum(1)], axis=1).astype(F)
    en1 = np.array([[labEf.sum()]], F)

    w1m = np.ascontiguousarray(W1.transpose(1, 2, 3, 0).reshape(27, 64).astype(BFH))
    w2t = np.ascontiguousarray(W2.transpose(1, 2, 3, 0).reshape(64, 9 * 128).astype(BFH))
    w3t = np.ascontiguousarray(W3.transpose(1, 2, 3, 0).reshape(128, 9 * 256).astype(BFH))
    wpt = np.ascontiguousarray(
        Wp.T.reshape(2, 128, 128).transpose(1, 0, 2).reshape(128, 256).astype(BFH))
    wcd = np.ascontiguousarray(wc_d.reshape(2, 128).T.astype(BFH))   # [128,2]
    bcd = np.array([[bc_d]], F)
    rxt = np.ascontiguousarray(Rx.T)                           # [40,320]

    shared = dict(w1m=w1m, w2t=w2t, w3t=w3t, wpt=wpt, wcd=wcd, bcd=bcd,
                  rxt=rxt, ebank=np.ascontiguousarray(ebank.astype(BFH)),
                  epack=epack, en1=en1)

    in_maps = []
    for c in range(8):
        b, half = c % 4, c // 4
        xs = _im2col(x_l[b], SUP_L1START[half], SUP_NL1).astype(BFH)
        xu = _im2col(x_ul[b], UL_L1START[half], UL_NL1).astype(BFH)
        f0 = SUP_FSTART[half]
        ryt = np.ascontiguousarray(
            Ry[160 * half:160 * half + 160, f0:f0 + SUP_NF].T)  # [21,160]
        y_h = y_l[b, 160 * half:160 * half + 160]                # [160,320]
        vmask = (y_h != 255).astype(F)
        yf = np.clip(y_h, 0, 1).astype(F)
        supA = vmask.astype(BFH)
        supB = (yf * vmask).astype(BFH)
        u0 = UL_FSTART[half]
        plm = (pl_ema[b, u0:u0 + 20] * mask_ema[b, u0:u0 + 20]).reshape(100, 8)
        mkm = mask_ema[b, u0:u0 + 20].reshape(100, 8)
        m = dict(shared)
        m.update(xs=xs, xu=xu, ryt=ryt, supA=np.ascontiguousarray(supA),
                 supB=np.ascontiguousarray(supB),
                 plm=np.ascontiguousarray(plm), mkm=np.ascontiguousarray(mkm))
        in_maps.append(m)

    meta = dict(sup_cnt=float((y_l != 255).sum()),
                cons_cnt=float(mask_ema.sum()),
                epoch=int(np.asarray(inputs['epoch'])))
    return in_maps, meta


def _combine(rows, meta):
    s = np.asarray(rows, np.float64).sum(axis=0)
    # slots: 0/1 sum softplus(z)*A, 2/3 sum z*B, 4 sum softplus(d_ul),
    # 5 sum d*sigmoid(d), 6 sum softplus(d)*mask, 7 sum d*plm
    sup_nll = (s[0] + s[1]) - (s[2] + s[3])
    ent = s[4] - s[5]
    cons = s[6] - s[7]
    contr_num, inc_sum = s[12], s[13]
    loss_sup = sup_nll / max(meta['sup_cnt'], 1.0)
    epoch = meta['epoch']
    if epoch < 5:
        return np.float32(loss_sup)
    loss_ent = ent / 6400.0
    loss_cons = cons / max(meta['cons_cnt'], 1.0)
    loss_contr = contr_num / max(inc_sum, 1.0)
    ramp = min(max(epoch / 40.0, 0.0), 1.0)
    cons_w = 1.0 * float(np.exp(-5.0 * (1.0 - ramp) ** 2))
    return np.float32(loss_sup + 0.1 * loss_contr + cons_w * loss_cons
                      + 0.01 * loss_ent)


# ---------------- bass program ----------------

def _build():
    import concourse.bacc as bacc
    import concourse.bass as bass
    import concourse.mybir as mybir
    from concourse import tile

    dt = mybir.dt
    F32 = dt.float32
    BF = dt.bfloat16
    AF = mybir.ActivationFunctionType
    OP = mybir.AluOpType

    nc = bacc.Bacc("TRN2", target_bir_lowering=False, debug=False,
                   num_devices=8)

    # ---- dram I/O ----
    din = {}
    for name, shape, dtp in [
            ('xs', [27, SUP_NL1 * 160], BF), ('xu', [27, UL_NL1 * 160], BF),
            ('ryt', [21, 160], F32), ('rxt', [40, 320], F32),
            ('supA', [160, 320], BF), ('supB', [160, 320], BF),
            ('plm', [100, 8], F32), ('mkm', [100, 8], F32),
            ('w1m', [27, 64], BF), ('w2t', [64, 9 * 128], BF),
            ('w3t', [128, 9 * 256], BF),
            ('wpt', [128, 256], BF), ('wcd', [128, 2], BF), ('bcd', [1, 1], F32),
            ('ebank', [128, 6400], BF), ('epack', [128, 2], F32),
            ('en1', [1, 1], F32)]:
        din[name] = nc.dram_tensor(name, shape, dtp, kind="ExternalInput")
    dout = nc.dram_tensor('part', [1, 16], F32, kind="ExternalOutput")

    eye_np = np.eye(128, dtype=np.float32)
    ones_np = np.ones((128, 128), dtype=np.float32)
    eye_d = nc.inline_tensor(eye_np, name='eye_c')
    ones_d = nc.inline_tensor(ones_np, name='ones_c')

    from contextlib import ExitStack
    with tile.TileContext(nc) as tc, ExitStack() as _es:
        cpool = _es.enter_context(tc.tile_pool(name="consts", bufs=1))
        big = _es.enter_context(tc.tile_pool(name="big", bufs=1))
        work = _es.enter_context(tc.tile_pool(name="work", bufs=1))
        wk2 = _es.enter_context(tc.tile_pool(name="wk2", bufs=2))
        dram = _es.enter_context(tc.tile_pool(name="dram", bufs=1, space="DRAM"))
        smallps = _es.enter_context(tc.tile_pool(name="smallps", bufs=2, space="PSUM"))

        # ---- consts / weights to SBUF ----
        def load(name, dtp=F32):
            src = din[name]
            t = cpool.tile(list(src.shape), dtp, name=f"{name}_sb")
            nc.sync.dma_start(t[:], src[:])
            return t

        eye_sb = cpool.tile([128, 128], F32, name="eye_sb")
        nc.sync.dma_start(eye_sb[:], eye_d[:])
        ones_sb = cpool.tile([128, 128], F32, name="ones_sb")
        nc.sync.dma_start(ones_sb[:], ones_d[:])
        ones_bf = cpool.tile([128, 1], BF, name="ones_bf")
        nc.vector.tensor_copy(ones_bf[:], ones_sb[:, 0:1])

        w1sb = load('w1m', BF)
        w2sb = load('w2t', BF)      # [64, tap*128]
        w3sb = load('w3t', BF)      # [128, tap*256]
        wpsb = load('wpt', BF)      # [128, half*128]
        wcsb = load('wcd', BF)
        bcdsb = load('bcd')
        rxsb = load('rxt')
        rysb = load('ryt')
        epsb = load('epack')
        en1sb = load('en1')
        plsb = load('plm')
        mksb = load('mkm')

        acc = work.tile([128, 12], F32, name="acc")
        nc.vector.memset(acc[:], 0.0)
        shiftsb = cpool.tile([128, 1], F32, name="shiftsb")
        nc.vector.memset(shiftsb[:], -SHIFT)
        eps24 = cpool.tile([1, 1], F32, name="eps24")
        nc.vector.memset(eps24[:], 1e-24)

        # bank [128, 12800] bf16: student (gathered) | EMA (host input).
        # EMA half DMAs start immediately - no deps.
        bank = big.tile([128, 12800], BF, name="bank")
        sbank = bank[:, 0:6400]
        ebsb = bank[:, 6400:12800]
        for i in range(4):
            nc.gpsimd.dma_start(ebsb[:, i * 1600:(i + 1) * 1600],
                                din['ebank'][:, i * 1600:(i + 1) * 1600])

        # ================= conv stack (bf16) =================
        def conv_stack(xdram, n_l1, n_l2, n_fea, tagp):
            with tc.tile_pool(name=f"convps{tagp}", bufs=4, space="PSUM") as cps:
                npix = n_l1 * 160
                l1t = big.tile([64, n_l1, 161], BF, tag="l1", name=f"l1{tagp}")
                nc.vector.memset(l1t[:, :, 160:161], 0.0)
                # conv1: stream im2col x through rotating chunk tiles;
                # relu alternates Act/DVE to keep both off the critical path
                xb = 0
                ck = 0
                while xb < npix:
                    xn = min(3840, npix - xb)
                    xt = big.tile([27, 3840], BF, tag="x", name=f"x{tagp}",
                                  bufs=2)
                    nc.sync.dma_start(xt[:, :xn], xdram[:, xb:xb + xn])
                    c0 = 0
                    while c0 < xn:
                        n = min(480, xn - c0)
                        nr = n // 160
                        g0 = (xb + c0) // 160
                        ps = cps.tile([64, 480], F32, tag="cps", name="ps1")
                        nc.tensor.matmul(ps[:, :n], w1sb[:],
                                         xt[:, c0:c0 + n],
                                         start=True, stop=True)
                        dst = l1t[:, g0: g0 + nr, 0:160]
                        src = ps[:, :n].rearrange("p (r x) -> p r x", x=160)
                        if ck % 2 == 0:
                            nc.scalar.activation(dst, src, AF.Relu)
                        else:
                            nc.vector.tensor_scalar_max(dst, src, 0.0)
                        ck += 1
                        c0 += n
                    xb += xn
                # conv2: 6-row chunks, 9 taps accumulate
                l2t = big.tile([128, n_l2, 81], BF, tag="l2", name=f"l2{tagp}")
                nc.vector.memset(l2t[:, :, 80:81], 0.0)
                r0 = 0
                while r0 < n_l2:
                    nr = min(6, n_l2 - r0)
                    ps = cps.tile([128, 480], F32, tag="cps", name="ps2")
                    pv = ps[:, :nr * 80].rearrange("p (r x) -> p r x", x=80)
                    for tap in range(9):
                        dy, dxx = tap // 3, tap % 3
                        rhs = l1t[:, 2 * r0 + dy: 2 * r0 + dy + 2 * nr - 1:2,
                                  dxx: dxx + 159:2]
                        nc.tensor.matmul(pv, w2sb[:, tap * 128:(tap + 1) * 128],
                                         rhs, start=(tap == 0), stop=(tap == 8))
                    nc.vector.tensor_scalar_max(l2t[:, r0:r0 + nr, 0:80], pv, 0.0)
                    r0 += nr
                # conv3: 12-row chunks, 2 M-halves, 9 taps
                feas = []
                for h in range(2):
                    ft = big.tile([128, n_fea * 40], BF, tag=f"fea{tagp}{h}",
                                  name=f"fea{tagp}{h}")
                    r0 = 0
                    while r0 < n_fea:
                        nr = min(12, n_fea - r0)
                        ps = cps.tile([128, 480], F32, tag="cps", name="ps3")
                        pv = ps[:, :nr * 40].rearrange("p (r x) -> p r x", x=40)
                        for tap in range(9):
                            dy, dxx = tap // 3, tap % 3
                            rhs = l2t[:, 2 * r0 + dy: 2 * r0 + dy + 2 * nr - 1:2,
                                      dxx: dxx + 79:2]
                            nc.tensor.matmul(
                                pv, w3sb[:, tap * 256 + h * 128:
                                         tap * 256 + h * 128 + 128],
                                rhs, start=(tap == 0), stop=(tap == 8))
                        nc.vector.tensor_scalar_max(
                            ft[:, r0 * 40:(r0 + nr) * 40],
                            ps[:, :nr * 40], 0.0)
                        r0 += nr
                    feas.append(ft)
            return feas

        with nc.named_scope("conv_ul"):
            fu_lo, fu_hi = conv_stack(din['xu'], UL_NL1, UL_NL2, UL_NF, "u")

        # ================= ul projection + payload + AllGather =================
        with nc.named_scope("proj"):
            proj_raw = work.tile([128, 800], F32, name="proj_raw")
            fuv = [t[:].rearrange("p (y x) -> p y x", x=40) for t in (fu_lo, fu_hi)]
            for ci in range(2):
                psp = smallps.tile([128, 400], F32, tag="sp", name="psp")
                for pp in range(4):
                    p = ci * 4 + pp
                    jl, kk = p // 4, p % 4
                    for h in range(2):
                        nc.tensor.matmul(
                            psp[:, pp * 100:(pp + 1) * 100],
                            wpsb[:, h * 128:(h + 1) * 128],
                            fuv[h][:, 10 * jl:10 * jl + 10,
                                   10 * kk:10 * kk + 10],
                            start=(h == 0), stop=(h == 1))
                nc.scalar.copy(proj_raw[:, ci * 400:(ci + 1) * 400], psp[:])
            # d_ul row (pixel-major) + labels
            du_row = work.tile([1, 800], F32, name="du_row")
            for ci in range(2):
                psd = smallps.tile([1, 400], F32, tag="sp", name="psd")
                for h in range(2):
                    nc.tensor.matmul(psd[:], wcsb[:, h:h + 1],
                                     (fu_lo, fu_hi)[h][:, ci * 400:(ci + 1) * 400],
                                     start=(h == 0), stop=(h == 1))
                nc.vector.tensor_scalar(du_row[:, ci * 400:(ci + 1) * 400],
                                        psd[:], bcdsb[0:1, 0:1], None,
                                        op0=OP.add)
            la_row = work.tile([1, 800], F32, name="la_row")
            nc.vector.tensor_scalar(la_row[:], du_row[:], 0.0, None, op0=OP.is_gt)
            # patch-major anchor labels
            la_pat = work.tile([1, 800], F32, name="la_pat")
            lrv = la_row[:].rearrange("p (y x) -> p y x", x=40)
            for p in range(8):
                jl, kk = p // 4, p % 4
                nc.sync.dma_start(
                    la_pat[:, p * 100:(p + 1) * 100].rearrange(
                        "p (y x) -> p y x", x=10),
                    lrv[:, 10 * jl:10 * jl + 10, 10 * kk:10 * kk + 10])
            # normalize proj: inv = exp(-0.5*ln(|a|^2 + eps))
            sq = work.tile([128, 800], F32, name="sq")
            nc.vector.tensor_mul(sq[:], proj_raw[:], proj_raw[:])
            nrm2 = work.tile([1, 800], F32, name="nrm2")
            for ci in range(2):
                pss = smallps.tile([1, 400], F32, tag="sp", name="pss")
                nc.tensor.matmul(pss[:], ones_sb[:, 0:1],
                                 sq[:, ci * 400:(ci + 1) * 400],
                                 start=True, stop=True)
                nc.scalar.copy(nrm2[:, ci * 400:(ci + 1) * 400], pss[:])
            lnx = work.tile([1, 800], F32, name="lnx")
            nc.scalar.activation(lnx[:], nrm2[:], AF.Ln, bias=eps24[:])
            inv = work.tile([1, 800], F32, name="inv")
            nc.scalar.activation(inv[:], lnx[:], AF.Exp, scale=-0.5)
            inv_bc = work.tile([128, 800], F32, tag="bc800", name="inv_bc")
            for ci in range(2):
                psb1 = smallps.tile([128, 400], F32, tag="sp", name="psb1")
                nc.tensor.matmul(psb1[:], ones_sb[0:1, :],
                                 inv[:, ci * 400:(ci + 1) * 400],
                                 start=True, stop=True)
                nc.scalar.copy(inv_bc[:, ci * 400:(ci + 1) * 400], psb1[:])
            proj_n = work.tile([128, 800], F32, name="proj_n")
            nc.vector.tensor_mul(proj_n[:], proj_raw[:], inv_bc[:])
            proj_nb = work.tile([128, 800], BF, name="proj_nb")
            nc.vector.tensor_copy(proj_nb[:], proj_n[:])
            la_bc = work.tile([128, 800], F32, tag="bc800", name="la_bc")
            for ci in range(2):
                psb2 = smallps.tile([128, 400], F32, tag="sp", name="psb2")
                nc.tensor.matmul(psb2[:], ones_sb[0:1, :],
                                 la_pat[:, ci * 400:(ci + 1) * 400],
                                 start=True, stop=True)
                nc.scalar.copy(la_bc[:, ci * 400:(ci + 1) * 400], psb2[:])
            vs2 = work.tile([128, 2], F32, name="vs2")
            nc.vector.tensor_reduce(vs2[:, 0:1], proj_n[:],
                                    mybir.AxisListType.X, OP.add)
            nc.vector.scalar_tensor_tensor(sq[:], proj_n[:], 1.0, la_bc[:],
                                           op0=OP.mult, op1=OP.mult,
                                           accum_out=vs2[:, 1:2])
            n1loc = work.tile([1, 1], F32, name="n1loc")
            nc.vector.tensor_reduce(n1loc[:], la_row[:], mybir.AxisListType.X,
                                    OP.add)

            # bf16 payload: [0:800] proj, f32 bits of (vsum,v1) at 800:804,
            # n1 at 804:806 (row 0), zero pad 806:808
            pay = dram.tile([128, 808], BF, name="pay")
            gath = dram.tile([1024, 808], BF, name="gath", addr_space="Shared")
            zpad = cpool.tile([128, 2], BF, name="zpad")
            nc.vector.memset(zpad[:], 0.0)
            nc.sync.dma_start(pay[:, 806:808], zpad[:])
            nc.sync.dma_start(pay[:, 0:800], proj_nb[:])
            nc.sync.dma_start(pay[:, 800:804].bitcast(F32), vs2[:])
            nc.sync.dma_start(pay[0:1, 804:806].bitcast(F32), n1loc[:])
            nc.gpsimd.collective_compute(
                "AllGather", OP.bypass,
                replica_groups=[list(range(8))],
                ins=[pay[:].opt()], outs=[gath[:].opt()])

        # ================= sup conv + head (overlaps AllGather) =============
        with nc.named_scope("conv_sup"):
            fs_lo, fs_hi = conv_stack(din['xs'], SUP_NL1, SUP_NL2, SUP_NF, "s")

        with nc.named_scope("sup_head"):
            dsup = work.tile([1, 840], F32, name="dsup")
            for ci in range(2):
                psd2 = smallps.tile([1, 420], F32, tag="sp", name="psd2")
                for h in range(2):
                    nc.tensor.matmul(psd2[:], wcsb[:, h:h + 1],
                                     (fs_lo, fs_hi)[h][:, ci * 420:(ci + 1) * 420],
                                     start=(h == 0), stop=(h == 1))
                nc.vector.tensor_scalar(dsup[:, ci * 420:(ci + 1) * 420],
                                        psd2[:], bcdsb[0:1, 0:1], None,
                                        op0=OP.add)
            d_yx = work.tile([21, 40], F32, name="d_yx")
            nc.sync.dma_start(d_yx[:], dsup[:].rearrange("p (y x) -> p y x", x=40))
            pstr = smallps.tile([40, 21], F32, tag="sp", name="pstr")
            nc.tensor.transpose(pstr[:], d_yx[:], eye_sb[0:21, 0:21])
            dT = work.tile([40, 21], F32, name="dT")
            nc.scalar.copy(dT[:], pstr[:])
            pst1 = smallps.tile([21, 320], F32, tag="sp", name="pst1")
            nc.tensor.matmul(pst1[:], dT[:], rxsb[:], start=True, stop=True)
            tmp1 = work.tile([21, 320], F32, name="tmp1")
            nc.scalar.copy(tmp1[:], pst1[:])
            # z chunks + CE partials: softplus(z) = ln(1 + e^z)
            for ci, (p0, npp) in enumerate([(0, 128), (128, 32)]):
                psz = smallps.tile([128, 320], F32, tag="sp", name="psz")
                nc.tensor.matmul(psz[:npp, :], rysb[:, p0:p0 + npp],
                                 tmp1[:], start=True, stop=True)
                asb = wk2.tile([128, 320], BF, tag="ab", name="asb")
                bsb = wk2.tile([128, 320], BF, tag="ab", name="bsb")
                nc.sync.dma_start(asb[:npp, :], din['supA'][p0:p0 + npp, :])
                nc.sync.dma_start(bsb[:npp, :], din['supB'][p0:p0 + npp, :])
                et = wk2.tile([128, 320], F32, tag="sp2", name="et")
                nc.scalar.activation(et[:npp, :], psz[:npp, :], AF.Exp)
                ut = wk2.tile([128, 320], F32, tag="ut", name="ut")
                nc.scalar.activation(ut[:npp, :], et[:npp, :], AF.Ln,
                                     bias=ones_sb[0:npp, 0:1])
                jk = wk2.tile([128, 320], F32, tag="jk", name="jk")
                nc.vector.scalar_tensor_tensor(
                    jk[:npp, :], ut[:npp, :], 1.0, asb[:npp, :],
                    op0=OP.mult, op1=OP.mult,
                    accum_out=acc[0:npp, 0 + ci:1 + ci])
                nc.vector.scalar_tensor_tensor(
                    jk[:npp, :], psz[:npp, :], 1.0, bsb[:npp, :],
                    op0=OP.mult, op1=OP.mult,
                    accum_out=acc[0:npp, 2 + ci:3 + ci])

        # ================= ul head =================
        with nc.named_scope("ul_head"):
            dut = work.tile([100, 8], F32, name="dut")
            nc.sync.dma_start(dut[:], du_row[:].rearrange("p (a b) -> p a b", b=8))
            tu = work.tile([100, 8], F32, name="tu")
            nc.scalar.activation(tu[:], dut[:], AF.Exp)           # e^d
            uu = work.tile([100, 8], F32, name="uu")
            nc.scalar.activation(uu[:], tu[:], AF.Ln,             # softplus(d)
                                 bias=ones_sb[0:100, 0:1],
                                 accum_out=acc[0:100, 4:5])
            vu = work.tile([100, 8], F32, name="vu")
            nc.vector.tensor_scalar_add(vu[:], tu[:], 1.0)
            rv = work.tile([100, 8], F32, name="rv")
            nc.vector.reciprocal(rv[:], vu[:])
            sg = work.tile([100, 8], F32, name="sg")
            nc.vector.tensor_mul(sg[:], tu[:], rv[:])             # sigmoid(d)
            jk2 = work.tile([100, 8], F32, name="jk2")
            nc.vector.scalar_tensor_tensor(jk2[:], dut[:], 1.0, sg[:],
                                           op0=OP.mult, op1=OP.mult,
                                           accum_out=acc[0:100, 5:6])
            nc.vector.scalar_tensor_tensor(jk2[:], uu[:], 1.0, mksb[:],
                                           op0=OP.mult, op1=OP.mult,
                                           accum_out=acc[0:100, 6:7])
            nc.vector.scalar_tensor_tensor(jk2[:], dut[:], 1.0, plsb[:],
                                           op0=OP.mult, op1=OP.mult,
                                           accum_out=acc[0:100, 7:8])

        # ================= contrastive =================
        # chunk = 128 bank entries on partitions x 800 anchors on free dim.
        # EMA chunks (bank cols 6400:12800) run before the gather lands.
        with nc.named_scope("contrastive"), \
                tc.tile_pool(name="cps2", bufs=1, space="PSUM") as cps2:
            psZ = cps2.tile([1, 1024], F32, tag="z", name="psZ")

            def process_chunk(k, i, n_total):
                ps = cps2.tile([128, 1024], F32, tag="cs", name="ps_s", bufs=2)
                cols = bank[:, k * 128:(k + 1) * 128]
                nc.tensor.matmul(ps[:, 0:512], cols, proj_nb[:, 0:512],
                                 start=True, stop=True)
                nc.tensor.matmul(ps[:, 512:800], cols, proj_nb[:, 512:800],
                                 start=True, stop=True)
                E = wk2.tile([128, 800], BF, tag="E", name="E", bufs=2)
                nc.scalar.activation(E[:], ps[:, 0:800], AF.Exp,
                                     scale=INV_TEMP, bias=shiftsb[:])
                st, sp_ = (i == 0), (i == n_total - 1)
                nc.tensor.matmul(psZ[:, 0:512], ones_bf[:], E[:, 0:512],
                                 start=st, stop=sp_)
                nc.tensor.matmul(psZ[:, 512:800], ones_bf[:], E[:, 512:800],
                                 start=st, stop=sp_)

            for i in range(50):
                process_chunk(50 + i, i, 100)

            # ---- post-AllGather: bank stats + student half ----
            gf = gath[:].bitcast(F32)  # [1024, 404] view of the bf16 gather
            gfv = gf.rearrange("(r p) c -> p c r", p=128)   # [128, 404, 8]
            vvall = work.tile([128, 2, 8], F32, name="vvall")
            for c in range(2):
                nc.sync.dma_start(vvall[:, c:c + 1, :], gfv[:, 400 + c:401 + c, :])
            BB = work.tile([128, 2], F32, name="BB")
            nc.vector.tensor_reduce(BB[:], vvall[:], mybir.AxisListType.X,
                                    OP.add)
            nc.vector.tensor_add(BB[:], BB[:], epsb[:])
            BBb = work.tile([128, 2], BF, name="BBb")
            nc.vector.tensor_copy(BBb[:], BB[:])
            n1row = work.tile([1, 8], F32, name="n1row")
            nc.sync.dma_start(n1row[:], gfv[0:1, 402:403, :])
            n1t = work.tile([1, 1], F32, name="n1t")
            nc.vector.tensor_reduce(n1t[:], n1row[:], mybir.AxisListType.X,
                                    OP.add)
            nc.vector.tensor_add(n1t[:], n1t[:], en1sb[:])
            MN1v = work.tile([1, 1], F32, name="MN1v")
            nc.vector.tensor_scalar(MN1v[:], n1t[:], -1.0, M_BANK,
                                    op0=OP.mult, op1=OP.add)
            DNv = work.tile([1, 1], F32, name="DNv")
            nc.vector.tensor_scalar(DNv[:], n1t[:], 2.0, -M_BANK,
                                    op0=OP.mult, op1=OP.add)
            for r in range(8):
                nc.gpsimd.dma_start(sbank[:, r * 800:(r + 1) * 800],
                                    gath[r * 128:(r + 1) * 128, 0:800])

            for i in range(50):
                process_chunk(i, 50 + i, 100)

            # ---- per-anchor assembly (rows [1,800], partition 0 only) ----
            T0 = work.tile([1, 800], F32, name="T0")
            T1 = work.tile([1, 800], F32, name="T1")
            for j, Trow in ((0, T0), (1, T1)):
                pst = cps2.tile([128, 1024], F32, tag="cs", name=f"psT{j}",
                                bufs=2)
                nc.tensor.matmul(pst[0:1, 0:512], BBb[:, j:j + 1],
                                 proj_nb[:, 0:512], start=True, stop=True)
                nc.tensor.matmul(pst[0:1, 512:800], BBb[:, j:j + 1],
                                 proj_nb[:, 512:800], start=True, stop=True)
                nc.scalar.copy(Trow[:], pst[0:1, 0:800])
            Lv = work.tile([1, 800], F32, name="Lv")
            nc.scalar.activation(Lv[:], psZ[0:1, 0:800], AF.Ln)
            nc.vector.tensor_scalar_add(Lv[:], Lv[:], SHIFT)

            def rowtile(name):
                return work.tile([1, 800], F32, name=name)

            m1, U1, tm, m0, U0, pd, dd, PA = (
                rowtile(n) for n in
                ('m1', 'U1', 'tmr', 'm0', 'U0', 'pdr', 'ddr', 'PAr'))
            nc.vector.tensor_scalar(m1[:], Lv[:], n1t[0:1, 0:1], None,
                                    op0=OP.mult)
            nc.vector.scalar_tensor_tensor(U1[:], T1[:], INV_TEMP, m1[:],
                                           op0=OP.mult, op1=OP.subtract)
            nc.vector.tensor_sub(tm[:], T0[:], T1[:])
            nc.vector.tensor_scalar(m0[:], Lv[:], MN1v[0:1, 0:1], None,
                                    op0=OP.mult)
            nc.vector.scalar_tensor_tensor(U0[:], tm[:], INV_TEMP, m0[:],
                                           op0=OP.mult, op1=OP.subtract)
            nc.vector.tensor_sub(m1[:], U1[:], U0[:])           # d10 (reuse m1)
            nc.vector.scalar_tensor_tensor(pd[:], la_pat[:], 1.0, m1[:],
                                           op0=OP.mult, op1=OP.mult)
            nc.vector.tensor_add(pd[:], pd[:], U0[:])
            nc.vector.tensor_scalar(dd[:], la_pat[:], DNv[0:1, 0:1], None,
                                    op0=OP.mult)
            nc.vector.tensor_scalar(dd[:], dd[:], MN1v[0:1, 0:1], None,
                                    op0=OP.add)
            nc.vector.tensor_scalar_max(dd[:], dd[:], 1.0)
            nc.vector.reciprocal(tm[:], dd[:])                  # reuse tm
            nc.vector.tensor_mul(PA[:], pd[:], tm[:])

            # patch means + include mask
            def r8tile(name):
                return work.tile([1, 8], F32, name=name)

            pap, ps8, fc, inc, g1, g2, cn = (
                r8tile(n) for n in
                ('pap', 'ps8', 'fcr', 'incr', 'g1r', 'g2r', 'cnr'))
            nc.vector.tensor_reduce(pap[:],
                                    PA[:].rearrange("p (b a) -> p b a", a=100),
                                    mybir.AxisListType.X, OP.add)
            nc.vector.tensor_reduce(ps8[:], la_pat[:].rearrange(
                "p (b a) -> p b a", a=100), mybir.AxisListType.X, OP.add)
            nc.vector.tensor_scalar_mul(pap[:], pap[:], -0.01)
            nc.vector.tensor_scalar_mul(fc[:], ps8[:], 0.01)
            nc.vector.tensor_scalar(g1[:], fc[:], 0.1, None, op0=OP.is_gt)
            nc.vector.tensor_scalar(g2[:], fc[:], 0.9, None, op0=OP.is_lt)
            nc.vector.tensor_mul(inc[:], g1[:], g2[:])
            nc.vector.tensor_scalar(inc[:], inc[:], -1.0, 1.0,
                                    op0=OP.mult, op1=OP.add)
            nc.vector.tensor_mul(cn[:], inc[:], pap[:])
            outrow = work.tile([1, 16], F32, name="outrow")
            nc.vector.memset(outrow[:], 0.0)
            nc.vector.tensor_reduce(outrow[:, 12:13], cn,
                                    mybir.AxisListType.X, OP.add)
            nc.vector.tensor_reduce(outrow[:, 13:14], inc,
                                    mybir.AxisListType.X, OP.add)
            psacc = smallps.tile([1, 12], F32, tag="sp", name="psacc")
            nc.tensor.matmul(psacc[:], ones_sb[:, 0:1], acc[:],
                             start=True, stop=True)
            nc.scalar.copy(outrow[:, 0:12], psacc[:])
            nc.sync.dma_start(dout[:], outrow[:])

    nc.compile()
    return nc


def _get_nc():
    if 'nc' not in _CACHE:
        _CACHE['nc'] = _build()
    return _CACHE['nc']


def run_on_cores(inputs, trace=False):
    """Returns (scalar_loss, exec_time_ns_or_None)."""
    from concourse.bass_utils import run_bass_kernel_spmd
    in_maps, meta = _prep(inputs)
    nc = _get_nc()
    res = run_bass_kernel_spmd(nc, in_maps, core_ids=list(range(8)),
                               trace=trace)
    rows = [res.results[c]['part'][0] for c in range(8)]
    return _combine(rows, meta), res.exec_time_ns


def run_timed(inputs, reps=5):
    """Correctness + timing: jit once, pre-place inputs on devices, time
    repeated executions (min over reps approximates HW exec + dispatch)."""
    import time
    import jax
    import numpy as np_
    import concourse.mybir as mybir
    from jax.sharding import Mesh, PartitionSpec, NamedSharding
    from jax.experimental.shard_map import shard_map
    from concourse import bass2jax
    from concourse.bass2jax import _bass_exec_p, partition_id_tensor

    bass2jax.install_neuronx_cc_hook()
    in_maps, meta = _prep(inputs)
    nc = _get_nc()

    partition_name = nc.partition_id_tensor.name if nc.partition_id_tensor else None
    in_names, out_names, out_avals, zero_outs = [], [], [], []
    for alloc in nc.m.functions[0].allocations:
        if not isinstance(alloc, mybir.MemoryLocationSet):
            continue
        name = alloc.memorylocations[0].name
        if alloc.kind == "ExternalInput":
            if name != partition_name:
                in_names.append(name)
        elif alloc.kind == "ExternalOutput":
            out_names.append(name)
            shape = tuple(alloc.tensor_shape)
            dtype = mybir.dt.np(alloc.dtype)
            out_avals.append(jax.core.ShapedArray(shape, dtype))
            zero_outs.append(np_.zeros(shape, dtype))
    n_params = len(in_names)
    all_names = in_names + out_names + ([partition_name] if partition_name else [])

    def _body(*args):
        operands = list(args)
        if partition_name is not None:
            operands.append(partition_id_tensor())
        outs = _bass_exec_p.bind(
            *operands, out_avals=tuple(out_avals), in_names=tuple(all_names),
            out_names=tuple(out_names), lowering_input_output_aliases=(),
            sim_require_finite=True, sim_require_nnan=True, nc=nc)
        return tuple(outs)

    devices = jax.devices()[:8]
    mesh = Mesh(np_.asarray(devices), ("core",))
    spec = NamedSharding(mesh, PartitionSpec("core"))
    n_outs = len(out_names)
    sharded = jax.jit(
        shard_map(_body, mesh=mesh,
                  in_specs=(PartitionSpec("core"),) * (n_params + n_outs),
                  out_specs=(PartitionSpec("core"),) * n_outs,
                  check_rep=False),
        keep_unused=True)
    concat_in = [
        jax.device_put(np_.concatenate(
            [np_.asarray(in_maps[c][in_names[i]]) for c in range(8)], axis=0), spec)
        for i in range(n_params)]
    concat_zeros = [
        jax.device_put(np_.zeros((8 * z.shape[0], *z.shape[1:]), z.dtype), spec)
        for z in zero_outs]
    jax.block_until_ready(concat_in)

    times = []
    outs = None
    for _ in range(reps):
        t0 = time.perf_counter()
        outs = sharded(*concat_in, *concat_zeros)
        jax.block_until_ready(outs)
        times.append(time.perf_counter() - t0)
    oarr = np_.asarray(outs[out_names.index('part')]).reshape(8, *out_avals[0].shape)
    rows = [oarr[c][0] for c in range(8)]
    return _combine(rows, meta), times


def bench_chain_slope(inputs, n_small=32, n_big=256, reps=4):
    """Per-execution device time: chain N executions (each consumes the
    previous call's output buffers -> serialized on device), slope between
    n_small and n_big cancels the axon dispatch overhead."""
    import time
    import jax
    import numpy as np_
    import concourse.mybir as mybir
    from jax.sharding import Mesh, PartitionSpec, NamedSharding
    from jax.experimental.shard_map import shard_map
    from concourse import bass2jax
    from concourse.bass2jax import _bass_exec_p, partition_id_tensor

    bass2jax.install_neuronx_cc_hook()
    in_maps, meta = _prep(inputs)
    nc = _get_nc()
    pname = nc.partition_id_tensor.name if nc.partition_id_tensor else None
    in_names, out_names, out_avals, zero_outs = [], [], [], []
    for alloc in nc.m.functions[0].allocations:
        if not isinstance(alloc, mybir.MemoryLocationSet):
            continue
        name = alloc.memorylocations[0].name
        if alloc.kind == "ExternalInput":
            if name != pname:
                in_names.append(name)
        elif alloc.kind == "ExternalOutput":
            out_names.append(name)
            shape = tuple(alloc.tensor_shape)
            dtype = mybir.dt.np(alloc.dtype)
            out_avals.append(jax.core.ShapedArray(shape, dtype))
            zero_outs.append(np_.zeros(shape, dtype))
    n_params = len(in_names)
    all_names = in_names + out_names + ([pname] if pname else [])

    def _body(*args):
        operands = list(args)
        if pname:
            operands.append(partition_id_tensor())
        return tuple(_bass_exec_p.bind(
            *operands, out_avals=tuple(out_avals), in_names=tuple(all_names),
            out_names=tuple(out_names), lowering_input_output_aliases=(),
            sim_require_finite=True, sim_require_nnan=True, nc=nc))

    devices = jax.devices()[:8]
    mesh = Mesh(np_.asarray(devices), ("core",))
    spec = NamedSharding(mesh, PartitionSpec("core"))
    n_outs = len(out_names)
    sharded = jax.jit(shard_map(_body, mesh=mesh,
                                in_specs=(PartitionSpec("core"),) * (n_params + n_outs),
                                out_specs=(PartitionSpec("core"),) * n_outs,
                                check_rep=False), keep_unused=True)
    concat_in = [jax.device_put(np_.concatenate(
        [np_.asarray(in_maps[c][in_names[i]]) for c in range(8)], axis=0), spec)
        for i in range(n_params)]
    concat_zeros = [jax.device_put(
        np_.zeros((8 * z.shape[0], *z.shape[1:]), z.dtype), spec)
        for z in zero_outs]
    jax.block_until_ready(concat_in)

    def run_chain(N):
        z = list(concat_zeros)
        t0 = time.perf_counter()
        for _ in range(N):
            z = list(sharded(*concat_in, *z))
        jax.block_until_ready(z)
        return time.perf_counter() - t0

    run_chain(2)  # warm-up / compile
    mins = {}
    for N in (n_small, n_big):
        mins[N] = min(run_chain(N) for _ in range(reps))
    slope = (mins[n_big] - mins[n_small]) / (n_big - n_small)
    return slope, mins


def kernel(**inputs):
    out, _ = run_on_cores(inputs, trace=False)
    return out
